# revision 13
# baseline (speedup 1.0000x reference)
"""Trainium2 Bass kernel for nn_AttnAggregator (GNN message passing, 8 cores).

Data-parallel over queries: each of 8 NeuronCores owns 256 queries = 2560
segments = 20 windows of 128 segments. Neighbor lists per window are padded
to T_w 128-slot tiles (T_w shared across cores so the SPMD program is
identical).

Host prep (pure data layout, inside kernel()):
  emx  f16 [128, NT*256]  neighbor embeddings pre-permuted into the exact
                          SBUF streaming layout (slot p of tile t)
  qoh  f16 [14, NT*128]   per-slot query one-hot (window-relative)
  segl f16 [128, NT]      segment-local id per slot (255 = pad)
  srx  f32 [SPC, 512]     per-segment [s_emb | r_emb] rows, pre-masked;
                          written to out[:, 256:768] by DRAM->DRAM DMA
  plus small constants (W chunks in f16, transposed s/r embeddings for the
  on-device c-table, v broadcast, iota, masks).

Device per window:
  em16  <- stream emx              (HWDGE, fp16)
  emT16 <- xbar transpose of em16  (z-matmul needs h on partitions)
  z     = em16 @ W1 + c[q]         (PE fp16; c added via query one-hot
                                    matmul against on-device c-table)
  H     = tanh(z)                  (ACT, 4-tile batches, fp16 out)
  score = sum_h H*v                (DVE scalar_tensor_tensor + accum)
  e     = exp(score)               (ACT, per window, fp16)
  wm    = (iota == segl) * e       (GPSIMD tensor_scalar, fp16)
  agg  += wm.T @ em ; den += wm.T @ 1   (PE fp16 -> PSUM f32)
  out[:, 0:256]   = agg / (den + empty)  (DVE recip + ACT scale-copy)
  out[:, 256:768] = srx                  (DRAM->DRAM, no compute)

The c-table c[q] = s_emb[q] @ W2 + r_emb[q] @ W3 + b is computed on-device
once for all 256 queries (fp16 matmuls).
"""

import os
import sys

import numpy as np

H = 256
SEQ_LEN = 10
NCORES = 8
WIN = 128  # segments per window (PSUM partition dim)
KQW = 14   # max queries touched by one window (ceil(128/10)+1)


def _build_core_shard(c, nbr_ids, seg_ids, QPC, NW):
    """Slice this core's neighbors; per-window counts."""
    seg_lo = c * QPC * SEQ_LEN
    seg_hi = (c + 1) * QPC * SEQ_LEN
    lo = np.searchsorted(seg_ids, seg_lo, "left")
    hi = np.searchsorted(seg_ids, seg_hi, "left")
    segs = (seg_ids[lo:hi] - seg_lo).astype(np.int64)  # 0 .. SPC-1
    nbrs = nbr_ids[lo:hi].astype(np.int64)
    wb = [np.searchsorted(segs, w * WIN, "left") for w in range(NW + 1)]
    cnts = [wb[w + 1] - wb[w] for w in range(NW)]
    return segs, nbrs, wb, cnts


def kernel(s, r, nbr_ids, seg_ids, ent_embeds, rel_embeds, W_attn, b_attn, v_s):
    sys.path.insert(0, "/opt/trn_rl_repo")
    import concourse.bass as bass  # noqa: F401
    import concourse.tile as tile
    from concourse import bacc, mybir
    from concourse.bass_utils import run_bass_kernel_spmd
    from contextlib import ExitStack

    f32 = mybir.dt.float32
    f16 = mybir.dt.float16
    AF = mybir.ActivationFunctionType
    OP = mybir.AluOpType

    s = np.asarray(s)
    r = np.asarray(r)
    nbr_ids = np.asarray(nbr_ids)
    seg_ids = np.asarray(seg_ids)
    ent_embeds = np.ascontiguousarray(np.asarray(ent_embeds, dtype=np.float32))
    rel_embeds = np.ascontiguousarray(np.asarray(rel_embeds, dtype=np.float32))
    W_attn = np.asarray(W_attn, dtype=np.float32)
    b_attn = np.asarray(b_attn, dtype=np.float32)
    v_s = np.asarray(v_s, dtype=np.float32).reshape(-1)

    B = s.shape[0]
    NUM_SEG = B * SEQ_LEN
    QPC = B // NCORES
    SPC = QPC * SEQ_LEN
    NW = SPC // WIN

    ent16 = ent_embeds.astype(np.float16)

    # ---------------- host-side layout ----------------
    shards = [_build_core_shard(c, nbr_ids, seg_ids, QPC, NW) for c in range(NCORES)]
    T_w = [
        max(1, max(-(-shards[c][3][w] // 128) for c in range(NCORES)))
        for w in range(NW)
    ]
    tb = np.concatenate([[0], np.cumsum(T_w)]).astype(np.int64)  # tile base per window
    NT = int(tb[-1])
    TMAX = max(T_w)

    counts_all = np.bincount(seg_ids.astype(np.int64), minlength=NUM_SEG)

    # per-window query meta (identical across cores)
    QB = [(w * WIN) // SEQ_LEN for w in range(NW)]
    KQ = [((w + 1) * WIN - 1) // SEQ_LEN - QB[w] + 1 for w in range(NW)]
    assert all(QB[w] % 128 + KQ[w] <= 128 for w in range(NW)), "window crosses q-chunk"

    # selection one-hot: c_win[k, w, :] = c_all[(QB[w]+k) % 128, w//10, :]
    sel_mat = np.zeros((128, NW * KQW), dtype=np.float16)
    for w in range(NW):
        for k in range(KQ[w]):
            sel_mat[(QB[w] + k) % 128, w * KQW + k] = 1.0

    # 4-strip variants: tile t of window w maps to row strip 32*(t%4);
    # group column block gb[w] + t//4.
    NGW = [-(-tw // 4) for tw in T_w]          # groups per window
    gb = np.concatenate([[0], np.cumsum(NGW)]).astype(np.int64)
    NG = int(gb[-1])
    GMAX = max(NGW)
    sel4_mat = np.zeros((128, NW * 128), dtype=np.float16)
    for w in range(NW):
        for j in range(4):
            for k in range(KQ[w]):
                sel4_mat[(QB[w] + k) % 128, w * 128 + 32 * j + k] = 1.0

    in_maps = []
    for c in range(NCORES):
        segs, nbrs, wb, cnts = shards[c]

        em_idx = np.full((NT, 128), -1, dtype=np.int64)   # ent row per slot
        segl = np.full((NT, 128), 255.0, dtype=np.float32)  # seg-local per slot
        qrel = np.full((NT, 128), -1, dtype=np.int64)     # window-rel query per slot
        for w in range(NW):
            cnt = cnts[w]
            flat_lo = tb[w] * 128
            sl = slice(wb[w], wb[w + 1])
            idx_flat = np.arange(flat_lo, flat_lo + cnt)
            em_idx.reshape(-1)[idx_flat] = nbrs[sl]
            segl.reshape(-1)[idx_flat] = (segs[sl] - w * WIN).astype(np.float32)
            qrel.reshape(-1)[idx_flat] = segs[sl] // SEQ_LEN - QB[w]

        # emx: [128, NT*256] f16; partition p, tile t -> ent16[em_idx[t, p]]
        E = ent16[np.maximum(em_idx, 0)]           # [NT, 128, 256]
        E[em_idx < 0] = 0
        emx = np.ascontiguousarray(E.transpose(1, 0, 2).reshape(128, NT * H))

        # qoh4: [128, NG*128] f16 one-hot of qrel; tile t -> rows 32*(t%4),
        # column block gb[w] + t//4
        qoh4 = np.zeros((128, NG * 128), dtype=np.float16)
        for w in range(NW):
            for t in range(T_w[w]):
                qr_t = qrel[tb[w] + t]                      # [128]
                vmask = qr_t >= 0
                cols = (gb[w] + t // 4) * 128 + np.arange(128)
                qoh4[32 * (t % 4) + qr_t[vmask], cols[vmask]] = 1.0

        seglT = np.ascontiguousarray(segl.T)  # [128, NT]

        # per-segment [s_emb | r_emb] rows, masked (for out[:, 256:768])
        segq = np.arange(SPC) // SEQ_LEN + c * QPC
        mask = (counts_all[c * SPC : (c + 1) * SPC] > 0).astype(np.float32)[:, None]
        srx = np.empty((SPC, 2 * H), dtype=np.float32)
        srx[:, 0:H] = ent_embeds[s[segq]] * mask
        srx[:, H : 2 * H] = rel_embeds[r[segq]] * mask

        # c-table operands: transposed per-query s/r embeddings (f16)
        sq = s[c * QPC : (c + 1) * QPC].astype(np.int64)
        rq = r[c * QPC : (c + 1) * QPC].astype(np.int64)
        s_embT = np.ascontiguousarray(
            ent16[sq].T.reshape(2, 128, QPC).transpose(1, 0, 2)
        )  # [128(hp), 2(hc), QPC(q)]
        r_embT = np.ascontiguousarray(
            rel_embeds[rq].astype(np.float16).T.reshape(2, 128, QPC).transpose(1, 0, 2)
        )

        invw = np.ascontiguousarray(
            (counts_all[c * SPC : (c + 1) * SPC] == 0)
            .astype(np.float32)
            .reshape(NW, 128)
            .T
        )  # [128, NW] 1.0 where segment empty

        im = {
            "emx": emx,
            "qoh4": qoh4,
            "segl": seglT,
            "srx": srx,
            "wq1": W_attn[0:256].reshape(2, 128, H).transpose(1, 0, 2)
                   .astype(np.float16).copy(),
            "wq2": W_attn[256:512].reshape(2, 128, H).transpose(1, 0, 2)
                   .astype(np.float16).copy(),
            "wq3": W_attn[512:768].reshape(2, 128, H).transpose(1, 0, 2)
                   .astype(np.float16).copy(),
            "brow": b_attn.reshape(1, H).astype(np.float16).copy(),
            "sT": s_embT,
            "rT": r_embT,
            "sel": sel_mat,
            "sel4": sel4_mat,
            "onesr": np.ones((1, 128), dtype=np.float16),
            "onesc": np.ones((128, 2), dtype=np.float16),
            "vbc": np.tile(v_s.astype(np.float16), (128, 4, 1)).reshape(128, 4 * 256),
            "iota": np.tile(np.arange(128, dtype=np.float16), (128, 1)),
            "invw": invw,
        }
        in_maps.append(im)

    # ---------------- build the SPMD program ----------------
    print("[kernel] host prep done", flush=True)
    nc = bacc.Bacc("TRN2", target_bir_lowering=False, debug=False,
                   num_devices=NCORES)

    def din(name, shape, dt):
        return nc.dram_tensor(name, shape, dt, kind="ExternalInput").ap()

    emx_ap = din("emx", [128, NT * H], f16)
    qoh4_ap = din("qoh4", [128, NG * 128], f16)
    segl_ap = din("segl", [128, NT], f32)
    srx_ap = din("srx", [SPC, 2 * H], f32)
    wq1_ap = din("wq1", [128, 2, H], f16)
    wq2_ap = din("wq2", [128, 2, H], f16)
    wq3_ap = din("wq3", [128, 2, H], f16)
    brow_ap = din("brow", [1, H], f16)
    sT_ap = din("sT", [128, 2, QPC], f16)
    rT_ap = din("rT", [128, 2, QPC], f16)
    sel_ap = din("sel", [128, NW * KQW], f16)
    sel4_ap = din("sel4", [128, NW * 128], f16)
    onesr_ap = din("onesr", [1, 128], f16)
    onesc_ap = din("onesc", [128, 2], f16)
    vbc_ap = din("vbc", [128, 4 * H], f16)
    iota_ap = din("iota", [128, 128], f16)
    invw_ap = din("invw", [128, NW], f32)
    out_ap = nc.dram_tensor("out", [SPC, 3 * H], f32, kind="ExternalOutput").ap()

    WM_ENGINE = os.environ.get("KV2_WM", "dve")

    import time as _time
    _t0 = _time.time()
    with tile.TileContext(nc) as tc, ExitStack() as ctx:
        cons = ctx.enter_context(tc.tile_pool(name="cons", bufs=1))
        emp = ctx.enter_context(tc.tile_pool(name="emp", bufs=3))
        emq = ctx.enter_context(tc.tile_pool(name="emq", bufs=2))
        qp = ctx.enter_context(tc.tile_pool(name="qp", bufs=2))
        wk = ctx.enter_context(tc.tile_pool(name="wk", bufs=2))
        wmp = ctx.enter_context(tc.tile_pool(name="wmp", bufs=4))
        op = ctx.enter_context(tc.tile_pool(name="op", bufs=2))
        psz = ctx.enter_context(tc.tile_pool(name="psz", bufs=2, space="PSUM"))
        psa = ctx.enter_context(tc.tile_pool(name="psa", bufs=2, space="PSUM"))
        psd = ctx.enter_context(tc.tile_pool(name="psd", bufs=2, space="PSUM"))

        def cload(tag, shape, dt, ap):
            t = cons.tile(shape, dt, tag=tag)
            nc.sync.dma_start(t[:], ap[:])
            return t

        wq1 = cload("wq1", [128, 2, H], f16, wq1_ap)
        wq2 = cload("wq2", [128, 2, H], f16, wq2_ap)
        wq3 = cload("wq3", [128, 2, H], f16, wq3_ap)
        brow = cload("brow", [1, H], f16, brow_ap)
        sT = cload("sT", [128, 2, QPC], f16, sT_ap)
        rT = cload("rT", [128, 2, QPC], f16, rT_ap)
        sel = cload("sel", [128, NW * KQW], f16, sel_ap)
        sel4 = cload("sel4", [128, NW * 128], f16, sel4_ap)
        onesr = cload("onesr", [1, 128], f16, onesr_ap)
        onesc = cload("onesc", [128, 2], f16, onesc_ap)
        vbc4_flat = cload("vbc", [128, 4 * H], f16, vbc_ap)
        vbc4 = vbc4_flat.rearrange("p (t h) -> p t h", t=4)
        iota = cload("iota", [128, 128], f16, iota_ap)
        invw = cload("invw", [128, NW], f32, invw_ap)
        segl = cload("segl", [128, NT], f32, segl_ap)

        # ---- c-table: c_all[q, hout] = s_emb@W2 + r_emb@W3 + b (fp16) ----
        c_all = cons.tile([128, 2, H], f16, tag="c_all")
        for qc in range(2):
            cp = psz.tile([128, 4, H], f32, tag="z")
            qs = slice(qc * 128, (qc + 1) * 128)
            for hc in range(2):
                nc.tensor.matmul(cp[:, 0, :], sT[:, hc, qs], wq2[:, hc, :],
                                 start=(hc == 0), stop=False)
            for hc in range(2):
                nc.tensor.matmul(cp[:, 0, :], rT[:, hc, qs], wq3[:, hc, :],
                                 start=False, stop=False)
            nc.tensor.matmul(cp[:, 0, :], onesr[:], brow[:],
                             start=False, stop=True)
            nc.scalar.activation(c_all[:, qc, :], cp[:, 0, :], AF.Copy)

        # per-window c rows replicated at partition strips 0/32/64/96
        c_win4 = cons.tile([128, NW, H], f16, tag="c_win4")
        for w in range(NW):
            cwp = psz.tile([128, 4, H], f32, tag="z")
            nc.tensor.matmul(cwp[:, 0, :],
                             sel4[:, w * 128 : (w + 1) * 128],
                             c_all[:, QB[w] // 128, :], start=True, stop=True)
            nc.scalar.activation(c_win4[:, w, :], cwp[:, 0, :], AF.Copy)

        # ---- main loop over windows ----
        NW_RUN = int(os.environ.get("KERNEL_NWIN", str(NW)))
        for w in range(NW_RUN):
            TW = T_w[w]
            base = int(tb[w])
            qb0 = QB[w] % 128
            ch = QB[w] // 128
            kq = KQ[w]

            em16 = emp.tile([128, TMAX, H], f16, tag="em")
            emT16 = emq.tile([128, 2 * TMAX, 128], f16, tag="emT")
            halves = [(0, TW // 2), (TW // 2, TW)]
            for (lo, hi) in halves:
                if hi <= lo:
                    continue
                nc.sync.dma_start(
                    em16[:, lo:hi, :],
                    emx_ap[:, (base + lo) * H : (base + hi) * H])
                nc.scalar.dma_start(
                    emT16[:, 2 * lo : 2 * hi, :], em16[:, lo:hi, :],
                    transpose=True)
            ng_w = -(-TW // 4)
            gbw = int(gb[w])
            qoh_w = qp.tile([128, GMAX * 128], f16, tag="qoh")
            nc.sync.dma_start(qoh_w[:, 0 : ng_w * 128],
                              qoh4_ap[:, gbw * 128 : (gbw + ng_w) * 128])

            scores = wk.tile([128, TMAX], f32, tag="sc")
            ebuf = wk.tile([128, TMAX], f32, tag="eb")

            ngrp = ng_w
            for g in range(ngrp):
                t0 = g * 4
                nt = min(4, TW - t0)
                zp = psz.tile([128, 4, H], f32, tag="z")
                QPACK = os.environ.get("KV4_QPACK", "1") == "1"
                if QPACK:
                    for tg in range(nt):
                        t = t0 + tg
                        zps = zp[:, tg, :]
                        nc.tensor.matmul(zps, emT16[:, 2 * t, :], wq1[:, 0, :],
                                         start=(tg % 2 == 0), stop=False,
                                         skip_group_check=True)
                        nc.tensor.matmul(zps, emT16[:, 2 * t + 1, :],
                                         wq1[:, 1, :],
                                         start=False, stop=False,
                                         skip_group_check=True)
                    for tg in range(nt):
                        nc.tensor.matmul(zp[:, tg, :],
                                         qoh_w[32 * tg : 32 * tg + kq,
                                               g * 128 : (g + 1) * 128],
                                         c_win4[32 * tg : 32 * tg + kq, w, :],
                                         start=False, stop=True,
                                         skip_group_check=True,
                                         tile_position=(32 * tg, 0))
                else:
                    for tg in range(nt):
                        t = t0 + tg
                        zps = zp[:, tg, :]
                        nc.tensor.matmul(zps, emT16[:, 2 * t, :], wq1[:, 0, :],
                                         start=True, stop=False)
                        nc.tensor.matmul(zps, emT16[:, 2 * t + 1, :],
                                         wq1[:, 1, :],
                                         start=False, stop=False)
                        nc.tensor.matmul(zp[:, tg, :],
                                         qoh_w[32 * tg : 32 * tg + kq,
                                               g * 128 : (g + 1) * 128],
                                         c_win4[32 * tg : 32 * tg + kq, w, :],
                                         start=False, stop=True,
                                         tile_position=(32 * tg, 0))
                Hsb = wk.tile([128, 4, H], f16, tag="H")
                nc.scalar.activation(Hsb[:, 0:nt, :], zp[:, 0:nt, :], AF.Tanh)
                hv = wk.tile([128, 4, H], f16, tag="hv")
                nc.vector.tensor_tensor(hv[:, 0:nt, :], Hsb[:, 0:nt, :],
                                        vbc4[:, 0:nt, :], OP.mult)
                if os.environ.get("KV4_TREE", "1") == "1":
                    hv2 = wk.tile([128, 4, 128], f16, tag="hv2")
                    nc.vector.tensor_tensor(hv2[:, 0:nt, :],
                                            hv[:, 0:nt, 0:128],
                                            hv[:, 0:nt, 128:256], OP.add)
                    nc.vector.tensor_tensor(hv[:, 0:nt, 0:64],
                                            hv2[:, 0:nt, 0:64],
                                            hv2[:, 0:nt, 64:128], OP.add)
                    nc.vector.tensor_tensor(hv2[:, 0:nt, 0:32],
                                            hv[:, 0:nt, 0:32],
                                            hv[:, 0:nt, 32:64], OP.add)
                    nc.vector.reduce_sum(scores[:, t0 : t0 + nt],
                                         hv2[:, 0:nt, 0:32],
                                         axis=mybir.AxisListType.X)
                else:
                    for tg in range(nt):
                        t = t0 + tg
                        junk = wk.tile([128, H], f16, tag="junk")
                        nc.vector.tensor_scalar(
                            junk[:], hv[:, tg, :], 1.0, 0.0, op0=OP.mult,
                            op1=OP.add, accum_out=scores[:, t : t + 1])

            nc.scalar.activation(ebuf[:, 0:TW], scores[:, 0:TW], AF.Exp)

            agg = psa.tile([128, 256], f32, tag="agg")
            den = psd.tile([128, 2], f32, tag="den")
            for t in range(TW):
                wm = wmp.tile([128, 128], f16, tag="wm")
                eng = nc.gpsimd if WM_ENGINE == "gpsimd" else nc.vector
                eng.tensor_scalar(wm[:], iota[:],
                                  segl[:, base + t : base + t + 1],
                                  ebuf[:, t : t + 1],
                                  op0=OP.is_equal, op1=OP.mult)
                nc.tensor.matmul(agg[:], wm[:], em16[:, t, :],
                                 start=(t == 0), stop=(t == TW - 1))
                nc.tensor.matmul(den[:], wm[:], onesc[:],
                                 start=(t == 0), stop=(t == TW - 1))

            dtmp = wk.tile([128, 1], f32, tag="dtmp")
            nc.vector.tensor_add(dtmp[:], den[:, 0:1], invw[:, w : w + 1])
            dinv = wk.tile([128, 1], f32, tag="dinv")
            nc.vector.reciprocal(dinv[:], dtmp[:])

            out_sb = op.tile([128, 256], f32, tag="out")
            nc.scalar.activation(out_sb[:], agg[:], AF.Copy,
                                 scale=dinv[:])
            nc.sync.dma_start(out_ap[w * 128 : (w + 1) * 128, 0:256],
                              out_sb[:])

            if w % 5 == 2:
                rlo = (w - 2) * 128
                rhi = min(rlo + 5 * 128, SPC)
                nc.scalar.dma_start(out_ap[rlo:rhi, 256:768],
                                    srx_ap[rlo:rhi, :])

    print(f"[kernel] program built+scheduled in {_time.time()-_t0:.1f}s",
          flush=True)
    nc.compile()
    print("[kernel] bacc.compile done; launching", flush=True)

    if os.environ.get("KERNEL_SIM"):
        from concourse.bass_interp import CoreSim
        sim = CoreSim(nc, trace=False)
        for k, v in in_maps[0].items():
            sim.tensor(k)[:] = v
        sim.simulate(check_with_hw=False)
        print("[kernel] CoreSim passed", flush=True)
        import types
        res = types.SimpleNamespace(
            results=[{"out": np.array(sim.tensor("out"))} for _ in range(NCORES)],
            exec_time_ns=None)
        out = np.concatenate([res.results[c]["out"] for c in range(NCORES)], axis=0)
        return out.reshape(B, SEQ_LEN, 3 * H)

    trace = bool(int(os.environ.get("KERNEL_TRACE", "0")))
    if trace:
        _install_prof_hook()
    res = run_bass_kernel_spmd(nc, in_maps, list(range(NCORES)), trace=trace)
    if trace and res.exec_time_ns is not None:
        print(f"HW exec time: {res.exec_time_ns} ns")

    out = np.concatenate([res.results[c]["out"] for c in range(NCORES)], axis=0)
    return out.reshape(B, SEQ_LEN, 3 * H)


def _install_prof_hook():
    """Shim antenv.axon_hooks so trace=True can NTFF-profile under axon."""
    import contextlib
    import ctypes
    import types

    import antenv

    if "antenv.axon_hooks" in sys.modules:
        return
    so = "/opt/axon/libaxon_pjrt.so"
    lib = ctypes.CDLL(so)
    if not hasattr(lib, "axon_start_nrt_profile"):
        return
    lib.axon_start_nrt_profile.argtypes = [ctypes.POINTER(ctypes.c_int64),
                                           ctypes.c_size_t]
    lib.axon_start_nrt_profile.restype = ctypes.c_int64
    lib.axon_stop_nrt_profile.argtypes = [ctypes.c_char_p]
    lib.axon_stop_nrt_profile.restype = ctypes.c_int64

    @contextlib.contextmanager
    def _hook(output_dir, device_ids):
        import jax

        jax.devices()
        if device_ids:
            ids = (ctypes.c_int64 * len(device_ids))(*device_ids)
            rc = lib.axon_start_nrt_profile(ids, len(device_ids))
        else:
            rc = lib.axon_start_nrt_profile(None, 0)
        if rc != 0:
            raise RuntimeError(f"axon_start_nrt_profile rc={rc}")
        try:
            yield
        finally:
            n = lib.axon_stop_nrt_profile(str(output_dir).encode())
            print(f"profile: {n} file(s) written to {output_dir}",
                  file=sys.stderr)

    mod = types.ModuleType("antenv.axon_hooks")
    mod.get_axon_ntff_profile_hook = lambda: _hook
    mod.set_axon_ntff_profile_hook = lambda h: None
    sys.modules["antenv.axon_hooks"] = mod
    antenv.axon_hooks = mod


# revision 14
# speedup vs baseline: 1.1410x; 1.1410x over previous
"""Trainium2 Bass kernel for nn_AttnAggregator (GNN message passing, 8 cores).

Data-parallel over queries: each of 8 NeuronCores owns 256 queries = 2560
segments = 20 windows of 128 segments. Neighbor lists per window are padded
to T_w 128-slot tiles (T_w shared across cores so the SPMD program is
identical).

Host prep (pure data layout, inside kernel()):
  emx  f16 [128, NT*256]  neighbor embeddings pre-permuted into the exact
                          SBUF streaming layout (slot p of tile t)
  qoh  f16 [14, NT*128]   per-slot query one-hot (window-relative)
  segl f16 [128, NT]      segment-local id per slot (255 = pad)
  srx  f32 [SPC, 512]     per-segment [s_emb | r_emb] rows, pre-masked;
                          written to out[:, 256:768] by DRAM->DRAM DMA
  plus small constants (W chunks in f16, transposed s/r embeddings for the
  on-device c-table, v broadcast, iota, masks).

Device per window:
  em16  <- stream emx              (HWDGE, fp16)
  emT16 <- xbar transpose of em16  (z-matmul needs h on partitions)
  z     = em16 @ W1 + c[q]         (PE fp16; c added via query one-hot
                                    matmul against on-device c-table)
  H     = tanh(z)                  (ACT, 4-tile batches, fp16 out)
  score = sum_h H*v                (DVE scalar_tensor_tensor + accum)
  e     = exp(score)               (ACT, per window, fp16)
  wm    = (iota == segl) * e       (GPSIMD tensor_scalar, fp16)
  agg  += wm.T @ em ; den += wm.T @ 1   (PE fp16 -> PSUM f32)
  out[:, 0:256]   = agg / (den + empty)  (DVE recip + ACT scale-copy)
  out[:, 256:768] = srx                  (DRAM->DRAM, no compute)

The c-table c[q] = s_emb[q] @ W2 + r_emb[q] @ W3 + b is computed on-device
once for all 256 queries (fp16 matmuls).
"""

import os
import sys

import numpy as np

H = 256
SEQ_LEN = 10
NCORES = 8
WIN = 128  # segments per window (PSUM partition dim)
KQW = 14   # max queries touched by one window (ceil(128/10)+1)


def _build_core_shard(c, nbr_ids, seg_ids, QPC, NW):
    """Slice this core's neighbors; per-window counts."""
    seg_lo = c * QPC * SEQ_LEN
    seg_hi = (c + 1) * QPC * SEQ_LEN
    lo = np.searchsorted(seg_ids, seg_lo, "left")
    hi = np.searchsorted(seg_ids, seg_hi, "left")
    segs = (seg_ids[lo:hi] - seg_lo).astype(np.int64)  # 0 .. SPC-1
    nbrs = nbr_ids[lo:hi].astype(np.int64)
    wb = [np.searchsorted(segs, w * WIN, "left") for w in range(NW + 1)]
    cnts = [wb[w + 1] - wb[w] for w in range(NW)]
    return segs, nbrs, wb, cnts


def kernel(s, r, nbr_ids, seg_ids, ent_embeds, rel_embeds, W_attn, b_attn, v_s):
    sys.path.insert(0, "/opt/trn_rl_repo")
    import concourse.bass as bass  # noqa: F401
    import concourse.tile as tile
    from concourse import bacc, mybir
    from concourse.bass_utils import run_bass_kernel_spmd
    from contextlib import ExitStack

    f32 = mybir.dt.float32
    f16 = mybir.dt.float16
    AF = mybir.ActivationFunctionType
    OP = mybir.AluOpType

    s = np.asarray(s)
    r = np.asarray(r)
    nbr_ids = np.asarray(nbr_ids)
    seg_ids = np.asarray(seg_ids)
    ent_embeds = np.ascontiguousarray(np.asarray(ent_embeds, dtype=np.float32))
    rel_embeds = np.ascontiguousarray(np.asarray(rel_embeds, dtype=np.float32))
    W_attn = np.asarray(W_attn, dtype=np.float32)
    b_attn = np.asarray(b_attn, dtype=np.float32)
    v_s = np.asarray(v_s, dtype=np.float32).reshape(-1)

    B = s.shape[0]
    NUM_SEG = B * SEQ_LEN
    QPC = B // NCORES
    SPC = QPC * SEQ_LEN
    NW = SPC // WIN

    ent16 = ent_embeds.astype(np.float16)

    # ---------------- host-side layout ----------------
    shards = [_build_core_shard(c, nbr_ids, seg_ids, QPC, NW) for c in range(NCORES)]
    T_w = [
        max(1, max(-(-shards[c][3][w] // 128) for c in range(NCORES)))
        for w in range(NW)
    ]
    tb = np.concatenate([[0], np.cumsum(T_w)]).astype(np.int64)  # tile base per window
    NT = int(tb[-1])
    TMAX = max(T_w)

    counts_all = np.bincount(seg_ids.astype(np.int64), minlength=NUM_SEG)

    # per-window query meta (identical across cores)
    QB = [(w * WIN) // SEQ_LEN for w in range(NW)]
    KQ = [((w + 1) * WIN - 1) // SEQ_LEN - QB[w] + 1 for w in range(NW)]
    assert all(QB[w] % 128 + KQ[w] <= 128 for w in range(NW)), "window crosses q-chunk"

    # selection one-hot: c_win[k, w, :] = c_all[(QB[w]+k) % 128, w//10, :]
    sel_mat = np.zeros((128, NW * KQW), dtype=np.float16)
    for w in range(NW):
        for k in range(KQ[w]):
            sel_mat[(QB[w] + k) % 128, w * KQW + k] = 1.0

    # 4-strip variants: tile t of window w maps to row strip 32*(t%4);
    # group column block gb[w] + t//4.
    NGW = [-(-tw // 4) for tw in T_w]          # groups per window
    gb = np.concatenate([[0], np.cumsum(NGW)]).astype(np.int64)
    NG = int(gb[-1])
    GMAX = max(NGW)
    sel4_mat = np.zeros((128, NW * 128), dtype=np.float16)
    for w in range(NW):
        for j in range(4):
            for k in range(KQ[w]):
                sel4_mat[(QB[w] + k) % 128, w * 128 + 32 * j + k] = 1.0

    in_maps = []
    for c in range(NCORES):
        segs, nbrs, wb, cnts = shards[c]

        em_idx = np.full((NT, 128), -1, dtype=np.int64)   # ent row per slot
        segl = np.full((NT, 128), 255.0, dtype=np.float32)  # seg-local per slot
        qrel = np.full((NT, 128), -1, dtype=np.int64)     # window-rel query per slot
        for w in range(NW):
            cnt = cnts[w]
            flat_lo = tb[w] * 128
            sl = slice(wb[w], wb[w + 1])
            idx_flat = np.arange(flat_lo, flat_lo + cnt)
            em_idx.reshape(-1)[idx_flat] = nbrs[sl]
            segl.reshape(-1)[idx_flat] = (segs[sl] - w * WIN).astype(np.float32)
            qrel.reshape(-1)[idx_flat] = segs[sl] // SEQ_LEN - QB[w]

        # emx: [128, NT*256] f16; partition p, tile t -> ent16[em_idx[t, p]]
        E = ent16[np.maximum(em_idx, 0)]           # [NT, 128, 256]
        E[em_idx < 0] = 0
        emx = np.ascontiguousarray(E.transpose(1, 0, 2).reshape(128, NT * H))

        # qoh: [KQW, NT*128] f16 one-hot of qrel
        qoh = np.zeros((KQW, NT * 128), dtype=np.float16)
        qr = qrel.reshape(-1)
        valid = qr >= 0
        qoh[qr[valid], np.nonzero(valid)[0]] = 1.0

        seglT = np.ascontiguousarray(segl.T)  # [128, NT]

        # per-segment [s_emb | r_emb] rows, masked (for out[:, 256:768])
        segq = np.arange(SPC) // SEQ_LEN + c * QPC
        mask = (counts_all[c * SPC : (c + 1) * SPC] > 0).astype(np.float32)[:, None]
        srx = np.empty((SPC, 2 * H), dtype=np.float32)
        srx[:, 0:H] = ent_embeds[s[segq]] * mask
        srx[:, H : 2 * H] = rel_embeds[r[segq]] * mask

        # c-table operands: transposed per-query s/r embeddings (f16)
        sq = s[c * QPC : (c + 1) * QPC].astype(np.int64)
        rq = r[c * QPC : (c + 1) * QPC].astype(np.int64)
        s_embT = np.ascontiguousarray(
            ent16[sq].T.reshape(2, 128, QPC).transpose(1, 0, 2)
        )  # [128(hp), 2(hc), QPC(q)]
        r_embT = np.ascontiguousarray(
            rel_embeds[rq].astype(np.float16).T.reshape(2, 128, QPC).transpose(1, 0, 2)
        )

        invw = np.ascontiguousarray(
            (counts_all[c * SPC : (c + 1) * SPC] == 0)
            .astype(np.float32)
            .reshape(NW, 128)
            .T
        )  # [128, NW] 1.0 where segment empty

        im = {
            "emx": emx,
            "qoh": qoh,
            "segl": seglT,
            "srx": srx,
            "wq1": W_attn[0:256].reshape(2, 128, H).transpose(1, 0, 2)
                   .astype(np.float16).copy(),
            "wq2": W_attn[256:512].reshape(2, 128, H).transpose(1, 0, 2)
                   .astype(np.float16).copy(),
            "wq3": W_attn[512:768].reshape(2, 128, H).transpose(1, 0, 2)
                   .astype(np.float16).copy(),
            "brow": b_attn.reshape(1, H).astype(np.float16).copy(),
            "sT": s_embT,
            "rT": r_embT,
            "sel": sel_mat,
            "onesr": np.ones((1, 128), dtype=np.float16),
            "onesc": np.ones((128, 2), dtype=np.float16),
            "vbc": np.tile(v_s.astype(np.float16), (128, 4, 1)).reshape(128, 4 * 256),
            "iota": np.tile(np.arange(128, dtype=np.float16), (128, 1)),
            "invw": invw,
        }
        in_maps.append(im)

    # ---------------- build the SPMD program ----------------
    print("[kernel] host prep done", flush=True)
    nc = bacc.Bacc("TRN2", target_bir_lowering=False, debug=False,
                   num_devices=NCORES)

    def din(name, shape, dt):
        return nc.dram_tensor(name, shape, dt, kind="ExternalInput").ap()

    emx_ap = din("emx", [128, NT * H], f16)
    qoh_ap = din("qoh", [KQW, NT * 128], f16)
    segl_ap = din("segl", [128, NT], f32)
    srx_ap = din("srx", [SPC, 2 * H], f32)
    wq1_ap = din("wq1", [128, 2, H], f16)
    wq2_ap = din("wq2", [128, 2, H], f16)
    wq3_ap = din("wq3", [128, 2, H], f16)
    brow_ap = din("brow", [1, H], f16)
    sT_ap = din("sT", [128, 2, QPC], f16)
    rT_ap = din("rT", [128, 2, QPC], f16)
    sel_ap = din("sel", [128, NW * KQW], f16)
    onesr_ap = din("onesr", [1, 128], f16)
    onesc_ap = din("onesc", [128, 2], f16)
    vbc_ap = din("vbc", [128, 4 * H], f16)
    iota_ap = din("iota", [128, 128], f16)
    invw_ap = din("invw", [128, NW], f32)
    out_ap = nc.dram_tensor("out", [SPC, 3 * H], f32, kind="ExternalOutput").ap()

    WM_ENGINE = os.environ.get("KV2_WM", "dve")

    import time as _time
    _t0 = _time.time()
    with tile.TileContext(nc) as tc, ExitStack() as ctx:
        cons = ctx.enter_context(tc.tile_pool(name="cons", bufs=1))
        emp = ctx.enter_context(tc.tile_pool(name="emp", bufs=3))
        emq = ctx.enter_context(tc.tile_pool(name="emq", bufs=2))
        qp = ctx.enter_context(tc.tile_pool(name="qp", bufs=2))
        wk = ctx.enter_context(tc.tile_pool(name="wk", bufs=2))
        wmp = ctx.enter_context(tc.tile_pool(name="wmp", bufs=4))
        op = ctx.enter_context(tc.tile_pool(name="op", bufs=2))
        psz = ctx.enter_context(tc.tile_pool(name="psz", bufs=2, space="PSUM"))
        psa = ctx.enter_context(tc.tile_pool(name="psa", bufs=2, space="PSUM"))
        psd = ctx.enter_context(tc.tile_pool(name="psd", bufs=2, space="PSUM"))

        def cload(tag, shape, dt, ap):
            t = cons.tile(shape, dt, tag=tag)
            nc.sync.dma_start(t[:], ap[:])
            return t

        wq1 = cload("wq1", [128, 2, H], f16, wq1_ap)
        wq2 = cload("wq2", [128, 2, H], f16, wq2_ap)
        wq3 = cload("wq3", [128, 2, H], f16, wq3_ap)
        brow = cload("brow", [1, H], f16, brow_ap)
        sT = cload("sT", [128, 2, QPC], f16, sT_ap)
        rT = cload("rT", [128, 2, QPC], f16, rT_ap)
        sel = cload("sel", [128, NW * KQW], f16, sel_ap)
        onesr = cload("onesr", [1, 128], f16, onesr_ap)
        onesc = cload("onesc", [128, 2], f16, onesc_ap)
        vbc4_flat = cload("vbc", [128, 4 * H], f16, vbc_ap)
        vbc4 = vbc4_flat.rearrange("p (t h) -> p t h", t=4)
        iota = cload("iota", [128, 128], f16, iota_ap)
        invw = cload("invw", [128, NW], f32, invw_ap)
        segl = cload("segl", [128, NT], f32, segl_ap)

        # ---- c-table: c_all[q, hout] = s_emb@W2 + r_emb@W3 + b (fp16) ----
        c_all = cons.tile([128, 2, H], f16, tag="c_all")
        for qc in range(2):
            cp = psz.tile([128, 4, H], f32, tag="z")
            qs = slice(qc * 128, (qc + 1) * 128)
            for hc in range(2):
                nc.tensor.matmul(cp[:, 0, :], sT[:, hc, qs], wq2[:, hc, :],
                                 start=(hc == 0), stop=False)
            for hc in range(2):
                nc.tensor.matmul(cp[:, 0, :], rT[:, hc, qs], wq3[:, hc, :],
                                 start=False, stop=False)
            nc.tensor.matmul(cp[:, 0, :], onesr[:], brow[:],
                             start=False, stop=True)
            nc.scalar.activation(c_all[:, qc, :], cp[:, 0, :], AF.Copy)

        # per-window c rows at partition base 0 (matmul RHS must start at 0)
        c_win = cons.tile([KQW, NW, H], f16, tag="c_win")
        for w in range(NW):
            cwp = psz.tile([128, 4, H], f32, tag="z")
            nc.tensor.matmul(cwp[0:KQW, 0, :],
                             sel[:, w * KQW : (w + 1) * KQW],
                             c_all[:, QB[w] // 128, :], start=True, stop=True)
            nc.scalar.activation(c_win[:, w, :], cwp[0:KQW, 0, :], AF.Copy)

        # ---- main loop over windows ----
        NW_RUN = int(os.environ.get("KERNEL_NWIN", str(NW)))
        for w in range(NW_RUN):
            TW = T_w[w]
            base = int(tb[w])
            qb0 = QB[w] % 128
            ch = QB[w] // 128
            kq = KQ[w]

            em16 = emp.tile([128, TMAX, H], f16, tag="em")
            emT16 = emq.tile([128, 2 * TMAX, 128], f16, tag="emT")
            halves = [(0, TW // 2), (TW // 2, TW)]
            for (lo, hi) in halves:
                if hi <= lo:
                    continue
                nc.sync.dma_start(
                    em16[:, lo:hi, :],
                    emx_ap[:, (base + lo) * H : (base + hi) * H])
                nc.scalar.dma_start(
                    emT16[:, 2 * lo : 2 * hi, :], em16[:, lo:hi, :],
                    transpose=True)
            qoh_w = qp.tile([KQW, TMAX * 128], f16, tag="qoh")
            nc.sync.dma_start(qoh_w[:, 0 : TW * 128],
                              qoh_ap[:, base * 128 : (base + TW) * 128])

            scores = wk.tile([128, TMAX], f32, tag="sc")
            ebuf = wk.tile([128, TMAX], f32, tag="eb")

            ngrp = -(-TW // 4)
            for g in range(ngrp):
                t0 = g * 4
                nt = min(4, TW - t0)
                zp = psz.tile([128, 4, H], f32, tag="z")
                for tg in range(nt):
                    t = t0 + tg
                    zps = zp[:, tg, :]
                    nc.tensor.matmul(zps, emT16[:, 2 * t, :], wq1[:, 0, :],
                                     start=True, stop=False)
                    nc.tensor.matmul(zps, emT16[:, 2 * t + 1, :], wq1[:, 1, :],
                                     start=False, stop=False)
                    nc.tensor.matmul(zps,
                                     qoh_w[0:kq, t * 128 : (t + 1) * 128],
                                     c_win[0:kq, w, :],
                                     start=False, stop=True)
                Hsb = wk.tile([128, 4, H], f16, tag="H")
                nc.scalar.activation(Hsb[:, 0:nt, :], zp[:, 0:nt, :], AF.Tanh)
                hv = wk.tile([128, 4, H], f16, tag="hv")
                nc.vector.tensor_tensor(hv[:, 0:nt, :], Hsb[:, 0:nt, :],
                                        vbc4[:, 0:nt, :], OP.mult)
                if os.environ.get("KV4_TREE", "1") == "1":
                    hv2 = wk.tile([128, 4, 128], f16, tag="hv2")
                    nc.vector.tensor_tensor(hv2[:, 0:nt, :],
                                            hv[:, 0:nt, 0:128],
                                            hv[:, 0:nt, 128:256], OP.add)
                    nc.vector.tensor_tensor(hv[:, 0:nt, 0:64],
                                            hv2[:, 0:nt, 0:64],
                                            hv2[:, 0:nt, 64:128], OP.add)
                    nc.vector.tensor_tensor(hv2[:, 0:nt, 0:32],
                                            hv[:, 0:nt, 0:32],
                                            hv[:, 0:nt, 32:64], OP.add)
                    nc.vector.reduce_sum(scores[:, t0 : t0 + nt],
                                         hv2[:, 0:nt, 0:32],
                                         axis=mybir.AxisListType.X)
                else:
                    for tg in range(nt):
                        t = t0 + tg
                        junk = wk.tile([128, H], f16, tag="junk")
                        nc.vector.tensor_scalar(
                            junk[:], hv[:, tg, :], 1.0, 0.0, op0=OP.mult,
                            op1=OP.add, accum_out=scores[:, t : t + 1])

            nc.scalar.activation(ebuf[:, 0:TW], scores[:, 0:TW], AF.Exp)

            agg = psa.tile([128, 256], f32, tag="agg")
            den = psd.tile([128, 2], f32, tag="den")
            for t in range(TW):
                wm = wmp.tile([128, 128], f16, tag="wm")
                eng = nc.gpsimd if WM_ENGINE == "gpsimd" else nc.vector
                eng.tensor_scalar(wm[:], iota[:],
                                  segl[:, base + t : base + t + 1],
                                  ebuf[:, t : t + 1],
                                  op0=OP.is_equal, op1=OP.mult)
                nc.tensor.matmul(agg[:], wm[:], em16[:, t, :],
                                 start=(t == 0), stop=(t == TW - 1))
                nc.tensor.matmul(den[:], wm[:], onesc[:],
                                 start=(t == 0), stop=(t == TW - 1))

            dtmp = wk.tile([128, 1], f32, tag="dtmp")
            nc.vector.tensor_add(dtmp[:], den[:, 0:1], invw[:, w : w + 1])
            dinv = wk.tile([128, 1], f32, tag="dinv")
            nc.vector.reciprocal(dinv[:], dtmp[:])

            out_sb = op.tile([128, 256], f32, tag="out")
            nc.scalar.activation(out_sb[:], agg[:], AF.Copy,
                                 scale=dinv[:])
            nc.sync.dma_start(out_ap[w * 128 : (w + 1) * 128, 0:256],
                              out_sb[:])

            if w % 5 == 2:
                rlo = (w - 2) * 128
                rhi = min(rlo + 5 * 128, SPC)
                nc.scalar.dma_start(out_ap[rlo:rhi, 256:768],
                                    srx_ap[rlo:rhi, :])

    print(f"[kernel] program built+scheduled in {_time.time()-_t0:.1f}s",
          flush=True)
    nc.compile()
    print("[kernel] bacc.compile done; launching", flush=True)

    if os.environ.get("KERNEL_SIM"):
        from concourse.bass_interp import CoreSim
        sim = CoreSim(nc, trace=False)
        for k, v in in_maps[0].items():
            sim.tensor(k)[:] = v
        sim.simulate(check_with_hw=False)
        print("[kernel] CoreSim passed", flush=True)
        import types
        res = types.SimpleNamespace(
            results=[{"out": np.array(sim.tensor("out"))} for _ in range(NCORES)],
            exec_time_ns=None)
        out = np.concatenate([res.results[c]["out"] for c in range(NCORES)], axis=0)
        return out.reshape(B, SEQ_LEN, 3 * H)

    trace = bool(int(os.environ.get("KERNEL_TRACE", "0")))
    if trace:
        _install_prof_hook()
    res = run_bass_kernel_spmd(nc, in_maps, list(range(NCORES)), trace=trace)
    if trace and res.exec_time_ns is not None:
        print(f"HW exec time: {res.exec_time_ns} ns")

    out = np.concatenate([res.results[c]["out"] for c in range(NCORES)], axis=0)
    return out.reshape(B, SEQ_LEN, 3 * H)


def _install_prof_hook():
    """Shim antenv.axon_hooks so trace=True can NTFF-profile under axon."""
    import contextlib
    import ctypes
    import types

    import antenv

    if "antenv.axon_hooks" in sys.modules:
        return
    so = "/opt/axon/libaxon_pjrt.so"
    lib = ctypes.CDLL(so)
    if not hasattr(lib, "axon_start_nrt_profile"):
        return
    lib.axon_start_nrt_profile.argtypes = [ctypes.POINTER(ctypes.c_int64),
                                           ctypes.c_size_t]
    lib.axon_start_nrt_profile.restype = ctypes.c_int64
    lib.axon_stop_nrt_profile.argtypes = [ctypes.c_char_p]
    lib.axon_stop_nrt_profile.restype = ctypes.c_int64

    @contextlib.contextmanager
    def _hook(output_dir, device_ids):
        import jax

        jax.devices()
        if device_ids:
            ids = (ctypes.c_int64 * len(device_ids))(*device_ids)
            rc = lib.axon_start_nrt_profile(ids, len(device_ids))
        else:
            rc = lib.axon_start_nrt_profile(None, 0)
        if rc != 0:
            raise RuntimeError(f"axon_start_nrt_profile rc={rc}")
        try:
            yield
        finally:
            n = lib.axon_stop_nrt_profile(str(output_dir).encode())
            print(f"profile: {n} file(s) written to {output_dir}",
                  file=sys.stderr)

    mod = types.ModuleType("antenv.axon_hooks")
    mod.get_axon_ntff_profile_hook = lambda: _hook
    mod.set_axon_ntff_profile_hook = lambda h: None
    sys.modules["antenv.axon_hooks"] = mod
    antenv.axon_hooks = mod


# revision 15
# speedup vs baseline: 1.4968x; 1.3118x over previous
"""Trainium2 Bass kernel for nn_AttnAggregator (GNN message passing, 8 cores).

Data-parallel over queries: each of 8 NeuronCores owns 256 queries = 2560
segments = 20 windows of 128 segments. Neighbor lists per window are padded
to 128-slot tiles. Each core processes its windows sorted by tile count
(descending), so the SPMD-uniform per-position tile count T_j = max over
cores of similarly-ranked windows (minimal padding); the host unpermutes
the output rows.

Host prep (pure data layout, inside kernel()):
  emx  f16 [128, NT*264]  neighbor embeddings pre-permuted into the agg
                          streaming layout (264 = 256 em + ones col + pad)
  emxT f16 [128, NT*256]  the same embeddings pre-transposed (h on
                          partitions) for the z-matmul
  qoh  f16 [14, NT*128]   per-slot query one-hot (window-relative)
  segl f32 [128, NT]      segment-local id per slot (255 = pad)
  srx  f32 [SPC, 512]     per-segment [s_emb | r_emb] rows, pre-masked;
                          written to out[:, 256:768] by DRAM->DRAM DMA
  selA/selB f16           per-position query-row selectors for the c-table
  plus small constants (W chunks in f16, transposed s/r embeddings, v
  broadcast, iota, empty-segment masks).

Device per window position j:
  em16  <- stream emx   (agg layout, fp16, ones col at 256)
  emT16 <- stream emxT  (z layout)
  z     = em @ W1 + c[q]           (PE fp16; c via query one-hot matmul
                                    against the on-device c-table)
  H     = tanh(z)                  (ACT, 4-tile groups, fp16 out)
  score = sum_h H*v                (DVE: tensor_tensor mult + add-tree)
  e     = exp(score)               (ACT, per group)
  wm    = (iota == segl) * e       (DVE tensor_scalar, fp16)
  agg|den += wm.T @ [em|1]         (PE fp16, single N=257 matmul -> PSUM)
  out[:, 0:256]   = agg / (den + empty)  (DVE recip + ACT scale-copy)
  out[:, 256:768] = srx                  (DRAM->DRAM, no compute)

The c-table c[q] = s_emb[q] @ W2 + r_emb[q] @ W3 + b is computed on-device
once for all 256 queries (fp16 matmuls).
"""

import os
import sys

import numpy as np

H = 256
EMW = 264  # em row width in the agg layout: 256 em + 1 ones + 7 pad
SEQ_LEN = 10
NCORES = 8
WIN = 128  # segments per window (PSUM partition dim)
KQW = 14   # max queries touched by one window (ceil(128/10)+1)


def _build_core_shard(c, nbr_ids, seg_ids, QPC, NW):
    """Slice this core's neighbors; per-window counts."""
    seg_lo = c * QPC * SEQ_LEN
    seg_hi = (c + 1) * QPC * SEQ_LEN
    lo = np.searchsorted(seg_ids, seg_lo, "left")
    hi = np.searchsorted(seg_ids, seg_hi, "left")
    segs = (seg_ids[lo:hi] - seg_lo).astype(np.int64)  # 0 .. SPC-1
    nbrs = nbr_ids[lo:hi].astype(np.int64)
    wb = [np.searchsorted(segs, w * WIN, "left") for w in range(NW + 1)]
    cnts = [wb[w + 1] - wb[w] for w in range(NW)]
    return segs, nbrs, wb, cnts


def kernel(s, r, nbr_ids, seg_ids, ent_embeds, rel_embeds, W_attn, b_attn, v_s):
    sys.path.insert(0, "/opt/trn_rl_repo")
    import concourse.bass as bass  # noqa: F401
    import concourse.tile as tile
    from concourse import bacc, mybir
    from concourse.bass_utils import run_bass_kernel_spmd
    from contextlib import ExitStack

    f32 = mybir.dt.float32
    f16 = mybir.dt.float16
    AF = mybir.ActivationFunctionType
    OP = mybir.AluOpType

    s = np.asarray(s)
    r = np.asarray(r)
    nbr_ids = np.asarray(nbr_ids)
    seg_ids = np.asarray(seg_ids)
    ent_embeds = np.ascontiguousarray(np.asarray(ent_embeds, dtype=np.float32))
    rel_embeds = np.ascontiguousarray(np.asarray(rel_embeds, dtype=np.float32))
    W_attn = np.asarray(W_attn, dtype=np.float32)
    b_attn = np.asarray(b_attn, dtype=np.float32)
    v_s = np.asarray(v_s, dtype=np.float32).reshape(-1)

    B = s.shape[0]
    NUM_SEG = B * SEQ_LEN
    QPC = B // NCORES
    SPC = QPC * SEQ_LEN
    NW = SPC // WIN

    ent16 = ent_embeds.astype(np.float16)

    # ---------------- host-side layout ----------------
    shards = [_build_core_shard(c, nbr_ids, seg_ids, QPC, NW) for c in range(NCORES)]
    tc_cw = np.array(
        [[max(1, -(-shards[c][3][w] // 128)) for w in range(NW)]
         for c in range(NCORES)])
    # per-core processing order: windows sorted by tile count descending
    perm = [list(np.argsort(-tc_cw[c], kind="stable")) for c in range(NCORES)]
    T_j = [int(max(tc_cw[c][perm[c][j]] for c in range(NCORES)))
           for j in range(NW)]
    tb = np.concatenate([[0], np.cumsum(T_j)]).astype(np.int64)
    NT = int(tb[-1])
    TMAX = max(T_j)

    counts_all = np.bincount(seg_ids.astype(np.int64), minlength=NUM_SEG)

    QB = [(w * WIN) // SEQ_LEN for w in range(NW)]  # first query of window w
    KQ = [((w + 1) * WIN - 1) // SEQ_LEN - QB[w] + 1 for w in range(NW)]
    assert all(QB[w] % 128 + KQ[w] <= 128 for w in range(NW))

    in_maps = []
    for c in range(NCORES):
        segs, nbrs, wb, cnts = shards[c]

        em_idx = np.full((NT, 128), -1, dtype=np.int64)
        segl = np.full((NT, 128), 255.0, dtype=np.float32)
        qoh = np.zeros((KQW, NT * 128), dtype=np.float16)
        selA = np.zeros((128, NW * KQW), dtype=np.float16)
        selB = np.zeros((128, NW * KQW), dtype=np.float16)
        invw = np.zeros((128, NW), dtype=np.float32)

        for j in range(NW):
            w = perm[c][j]
            cnt = cnts[w]
            flat_lo = int(tb[j]) * 128
            sl = slice(wb[w], wb[w + 1])
            idx_flat = np.arange(flat_lo, flat_lo + cnt)
            em_idx.reshape(-1)[idx_flat] = nbrs[sl]
            segl.reshape(-1)[idx_flat] = (segs[sl] - w * WIN).astype(np.float32)
            qr = segs[sl] // SEQ_LEN - QB[w]
            qoh[qr, idx_flat] = 1.0
            selw = selA if QB[w] < 128 else selB
            for k in range(KQ[w]):
                selw[(QB[w] + k) % 128, j * KQW + k] = 1.0
            invw[:, j] = (
                counts_all[c * SPC + w * WIN : c * SPC + (w + 1) * WIN] == 0
            ).astype(np.float32)

        E = ent16[np.maximum(em_idx, 0)]           # [NT, 128, 256]
        E[em_idx < 0] = 0
        emx = np.zeros((NT, 128, EMW), dtype=np.float16)
        emx[:, :, 0:H] = E
        emx[:, :, H] = 1.0
        emx = np.ascontiguousarray(emx.transpose(1, 0, 2).reshape(128, NT * EMW))
        emxT = np.ascontiguousarray(
            E.reshape(NT, 128, 2, 128).transpose(3, 0, 2, 1).reshape(128, NT * H))

        seglT = np.ascontiguousarray(segl.T)  # [128, NT]

        # per-segment [s_emb | r_emb] rows, masked (original row order)
        segq = np.arange(SPC) // SEQ_LEN + c * QPC
        mask = (counts_all[c * SPC : (c + 1) * SPC] > 0).astype(np.float32)[:, None]
        srx = np.empty((SPC, 2 * H), dtype=np.float32)
        srx[:, 0:H] = ent_embeds[s[segq]] * mask
        srx[:, H : 2 * H] = rel_embeds[r[segq]] * mask

        sq = s[c * QPC : (c + 1) * QPC].astype(np.int64)
        rq = r[c * QPC : (c + 1) * QPC].astype(np.int64)
        s_embT = np.ascontiguousarray(
            ent16[sq].T.reshape(2, 128, QPC).transpose(1, 0, 2))
        r_embT = np.ascontiguousarray(
            rel_embeds[rq].astype(np.float16).T.reshape(2, 128, QPC)
            .transpose(1, 0, 2))

        im = {
            "emx": emx,
            "emxT": emxT,
            "qoh": qoh,
            "segl": seglT,
            "srx": srx,
            "wq1": W_attn[0:256].reshape(2, 128, H).transpose(1, 0, 2)
                   .astype(np.float16).copy(),
            "wq2": W_attn[256:512].reshape(2, 128, H).transpose(1, 0, 2)
                   .astype(np.float16).copy(),
            "wq3": W_attn[512:768].reshape(2, 128, H).transpose(1, 0, 2)
                   .astype(np.float16).copy(),
            "brow": b_attn.reshape(1, H).astype(np.float16).copy(),
            "sT": s_embT,
            "rT": r_embT,
            "selA": selA,
            "selB": selB,
            "onesr": np.ones((1, 128), dtype=np.float16),
            "vbc": np.tile(v_s.astype(np.float16), (128, 4, 1))
                   .reshape(128, 4 * H),
            "iota": np.tile(np.arange(128, dtype=np.float16), (128, 1)),
            "invw": invw,
        }
        in_maps.append(im)

    # ---------------- build the SPMD program ----------------
    print("[kernel] host prep done", flush=True)
    nc = bacc.Bacc("TRN2", target_bir_lowering=False, debug=False,
                   num_devices=NCORES)

    def din(name, shape, dt):
        return nc.dram_tensor(name, shape, dt, kind="ExternalInput").ap()

    emx_ap = din("emx", [128, NT * EMW], f16)
    emxT_ap = din("emxT", [128, NT * H], f16)
    qoh_ap = din("qoh", [KQW, NT * 128], f16)
    segl_ap = din("segl", [128, NT], f32)
    srx_ap = din("srx", [SPC, 2 * H], f32)
    wq1_ap = din("wq1", [128, 2, H], f16)
    wq2_ap = din("wq2", [128, 2, H], f16)
    wq3_ap = din("wq3", [128, 2, H], f16)
    brow_ap = din("brow", [1, H], f16)
    sT_ap = din("sT", [128, 2, QPC], f16)
    rT_ap = din("rT", [128, 2, QPC], f16)
    selA_ap = din("selA", [128, NW * KQW], f16)
    selB_ap = din("selB", [128, NW * KQW], f16)
    onesr_ap = din("onesr", [1, 128], f16)
    vbc_ap = din("vbc", [128, 4 * H], f16)
    iota_ap = din("iota", [128, 128], f16)
    invw_ap = din("invw", [128, NW], f32)
    out_ap = nc.dram_tensor("out", [SPC, 3 * H], f32, kind="ExternalOutput").ap()

    import time as _time
    _t0 = _time.time()
    with tile.TileContext(nc) as tc, ExitStack() as ctx:
        cons = ctx.enter_context(tc.tile_pool(name="cons", bufs=1))
        emp = ctx.enter_context(tc.tile_pool(name="emp", bufs=3))
        emq = ctx.enter_context(tc.tile_pool(name="emq", bufs=3))
        qp = ctx.enter_context(tc.tile_pool(name="qp", bufs=2))
        wk = ctx.enter_context(tc.tile_pool(name="wk", bufs=2))
        wmp = ctx.enter_context(tc.tile_pool(name="wmp", bufs=4))
        op = ctx.enter_context(tc.tile_pool(name="op", bufs=2))
        psz = ctx.enter_context(tc.tile_pool(name="psz", bufs=2, space="PSUM"))
        psa = ctx.enter_context(tc.tile_pool(name="psa", bufs=2, space="PSUM"))

        def cload(tag, shape, dt, ap):
            t = cons.tile(shape, dt, tag=tag)
            nc.sync.dma_start(t[:], ap[:])
            return t

        wq1 = cload("wq1", [128, 2, H], f16, wq1_ap)
        wq2 = cload("wq2", [128, 2, H], f16, wq2_ap)
        wq3 = cload("wq3", [128, 2, H], f16, wq3_ap)
        brow = cload("brow", [1, H], f16, brow_ap)
        sT = cload("sT", [128, 2, QPC], f16, sT_ap)
        rT = cload("rT", [128, 2, QPC], f16, rT_ap)
        selA = cload("selA", [128, NW * KQW], f16, selA_ap)
        selB = cload("selB", [128, NW * KQW], f16, selB_ap)
        onesr = cload("onesr", [1, 128], f16, onesr_ap)
        vbc4_flat = cload("vbc", [128, 4 * H], f16, vbc_ap)
        vbc4 = vbc4_flat.rearrange("p (t h) -> p t h", t=4)
        iota = cload("iota", [128, 128], f16, iota_ap)
        invw = cload("invw", [128, NW], f32, invw_ap)
        segl = cload("segl", [128, NT], f32, segl_ap)

        # ---- c-table: c_all[q, hout] = s_emb@W2 + r_emb@W3 + b (fp16) ----
        c_all = cons.tile([128, 2, H], f16, tag="c_all")
        for qc in range(2):
            cp = psz.tile([128, 4, H], f32, tag="z")
            qs = slice(qc * 128, (qc + 1) * 128)
            for hc in range(2):
                nc.tensor.matmul(cp[:, 0, :], sT[:, hc, qs], wq2[:, hc, :],
                                 start=(hc == 0), stop=False)
            for hc in range(2):
                nc.tensor.matmul(cp[:, 0, :], rT[:, hc, qs], wq3[:, hc, :],
                                 start=False, stop=False)
            nc.tensor.matmul(cp[:, 0, :], onesr[:], brow[:],
                             start=False, stop=True)
            nc.scalar.activation(c_all[:, qc, :], cp[:, 0, :], AF.Copy)

        # per-position c rows at partition base 0 (both query chunks)
        c_win = cons.tile([KQW, NW, H], f16, tag="c_win")
        for j in range(NW):
            cwp = psz.tile([128, 4, H], f32, tag="z")
            nc.tensor.matmul(cwp[0:KQW, 0, :],
                             selA[:, j * KQW : (j + 1) * KQW],
                             c_all[:, 0, :], start=True, stop=False)
            nc.tensor.matmul(cwp[0:KQW, 0, :],
                             selB[:, j * KQW : (j + 1) * KQW],
                             c_all[:, 1, :], start=False, stop=True)
            nc.scalar.activation(c_win[:, j, :], cwp[0:KQW, 0, :], AF.Copy)

        # ---- main loop over window positions ----
        NW_RUN = int(os.environ.get("KERNEL_NWIN", str(NW)))
        for j in range(NW_RUN):
            TW = T_j[j]
            base = int(tb[j])

            em16 = emp.tile([128, TMAX, EMW], f16, tag="em")
            emT16 = emq.tile([128, 2 * TMAX, 128], f16, tag="emT")
            halves = [(0, TW // 2), (TW // 2, TW)]
            for (lo, hi) in halves:
                if hi <= lo:
                    continue
                nc.sync.dma_start(
                    em16[:, lo:hi, :],
                    emx_ap[:, (base + lo) * EMW : (base + hi) * EMW])
                nc.scalar.dma_start(
                    emT16[:, 2 * lo : 2 * hi, :],
                    emxT_ap[:, (base + lo) * H : (base + hi) * H])
            qoh_w = qp.tile([KQW, TMAX * 128], f16, tag="qoh")
            nc.scalar.dma_start(qoh_w[:, 0 : TW * 128],
                                qoh_ap[:, base * 128 : (base + TW) * 128])

            scores = wk.tile([128, TMAX], f32, tag="sc")
            ebuf = wk.tile([128, TMAX], f32, tag="eb")

            ngrp = -(-TW // 4)
            for g in range(ngrp):
                t0 = g * 4
                nt = min(4, TW - t0)
                zp = psz.tile([128, 4, H], f32, tag="z")
                for tg in range(nt):
                    t = t0 + tg
                    zps = zp[:, tg, :]
                    nc.tensor.matmul(zps, emT16[:, 2 * t, :], wq1[:, 0, :],
                                     start=True, stop=False)
                    nc.tensor.matmul(zps, emT16[:, 2 * t + 1, :], wq1[:, 1, :],
                                     start=False, stop=False)
                    nc.tensor.matmul(zps,
                                     qoh_w[0:KQW, t * 128 : (t + 1) * 128],
                                     c_win[0:KQW, j, :],
                                     start=False, stop=True)
                Hsb = wk.tile([128, 4, H], f16, tag="H")
                nc.scalar.activation(Hsb[:, 0:nt, :], zp[:, 0:nt, :], AF.Tanh)
                hv = wk.tile([128, 4, H], f16, tag="hv")
                hv2 = wk.tile([128, 4, 128], f16, tag="hv2")
                nc.vector.tensor_tensor(hv[:, 0:nt, :], Hsb[:, 0:nt, :],
                                        vbc4[:, 0:nt, :], OP.mult)
                nc.vector.tensor_tensor(hv2[:, 0:nt, :], hv[:, 0:nt, 0:128],
                                        hv[:, 0:nt, 128:256], OP.add)
                nc.vector.tensor_tensor(hv[:, 0:nt, 0:64], hv2[:, 0:nt, 0:64],
                                        hv2[:, 0:nt, 64:128], OP.add)
                nc.vector.tensor_tensor(hv2[:, 0:nt, 0:32], hv[:, 0:nt, 0:32],
                                        hv[:, 0:nt, 32:64], OP.add)
                nc.vector.reduce_sum(scores[:, t0 : t0 + nt],
                                     hv2[:, 0:nt, 0:32],
                                     axis=mybir.AxisListType.X)
                nc.scalar.activation(ebuf[:, t0 : t0 + nt],
                                     scores[:, t0 : t0 + nt], AF.Exp)

            agg = psa.tile([128, EMW], f32, tag="agg")
            for t in range(TW):
                wm = wmp.tile([128, 128], f16, tag="wm")
                nc.vector.tensor_scalar(wm[:], iota[:],
                                        segl[:, base + t : base + t + 1],
                                        ebuf[:, t : t + 1],
                                        op0=OP.is_equal, op1=OP.mult)
                nc.tensor.matmul(agg[:, 0:257], wm[:], em16[:, t, 0:257],
                                 start=(t == 0), stop=(t == TW - 1))

            dtmp = wk.tile([128, 1], f32, tag="dtmp")
            nc.vector.tensor_add(dtmp[:], agg[:, 256:257], invw[:, j : j + 1])
            dinv = wk.tile([128, 1], f32, tag="dinv")
            nc.vector.reciprocal(dinv[:], dtmp[:])

            out_sb = op.tile([128, 256], f32, tag="out")
            nc.scalar.activation(out_sb[:], agg[:, 0:256], AF.Copy,
                                 scale=dinv[:])
            nc.sync.dma_start(out_ap[j * 128 : (j + 1) * 128, 0:256],
                              out_sb[:])

            if j % 5 == 2:
                rlo = (j - 2) * 128
                rhi = min(rlo + 5 * 128, SPC)
                nc.scalar.dma_start(out_ap[rlo:rhi, 256:768],
                                    srx_ap[rlo:rhi, :])

    print(f"[kernel] program built+scheduled in {_time.time()-_t0:.1f}s",
          flush=True)
    nc.compile()
    print("[kernel] bacc.compile done; launching", flush=True)

    def assemble(core_outs):
        full = np.empty((NCORES * SPC, 3 * H), dtype=np.float32)
        for c in range(NCORES):
            o = core_outs[c]
            blk = full[c * SPC : (c + 1) * SPC]
            blk[:, 256:768] = o[:, 256:768]
            for j in range(NW):
                w = perm[c][j]
                blk[w * WIN : (w + 1) * WIN, 0:256] = \
                    o[j * WIN : (j + 1) * WIN, 0:256]
        return full.reshape(B, SEQ_LEN, 3 * H)

    if os.environ.get("KERNEL_SIM"):
        from concourse.bass_interp import CoreSim
        sim = CoreSim(nc, trace=False)
        for k, v in in_maps[0].items():
            sim.tensor(k)[:] = v
        sim.simulate(check_with_hw=False)
        print("[kernel] CoreSim passed", flush=True)
        return assemble([np.array(sim.tensor("out"))] * NCORES)

    trace = bool(int(os.environ.get("KERNEL_TRACE", "0")))
    if trace:
        _install_prof_hook()
    res = run_bass_kernel_spmd(nc, in_maps, list(range(NCORES)), trace=trace)
    if trace and res.exec_time_ns is not None:
        print(f"HW exec time: {res.exec_time_ns} ns")

    return assemble([res.results[c]["out"] for c in range(NCORES)])


def _install_prof_hook():
    """Shim antenv.axon_hooks so trace=True can NTFF-profile under axon."""
    import contextlib
    import ctypes
    import types

    import antenv

    if "antenv.axon_hooks" in sys.modules:
        return
    so = "/opt/axon/libaxon_pjrt.so"
    lib = ctypes.CDLL(so)
    if not hasattr(lib, "axon_start_nrt_profile"):
        return
    lib.axon_start_nrt_profile.argtypes = [ctypes.POINTER(ctypes.c_int64),
                                           ctypes.c_size_t]
    lib.axon_start_nrt_profile.restype = ctypes.c_int64
    lib.axon_stop_nrt_profile.argtypes = [ctypes.c_char_p]
    lib.axon_stop_nrt_profile.restype = ctypes.c_int64

    @contextlib.contextmanager
    def _hook(output_dir, device_ids):
        import jax

        jax.devices()
        if device_ids:
            ids = (ctypes.c_int64 * len(device_ids))(*device_ids)
            rc = lib.axon_start_nrt_profile(ids, len(device_ids))
        else:
            rc = lib.axon_start_nrt_profile(None, 0)
        if rc != 0:
            raise RuntimeError(f"axon_start_nrt_profile rc={rc}")
        try:
            yield
        finally:
            n = lib.axon_stop_nrt_profile(str(output_dir).encode())
            print(f"profile: {n} file(s) written to {output_dir}",
                  file=sys.stderr)

    mod = types.ModuleType("antenv.axon_hooks")
    mod.get_axon_ntff_profile_hook = lambda: _hook
    mod.set_axon_ntff_profile_hook = lambda h: None
    sys.modules["antenv.axon_hooks"] = mod
    antenv.axon_hooks = mod


# revision 16
# speedup vs baseline: 1.8645x; 1.2457x over previous
"""Trainium2 Bass kernel for nn_AttnAggregator (GNN message passing, 8 cores).

Data-parallel over queries: each of 8 NeuronCores owns 256 queries = 2560
segments = 20 windows of 128 segments. Neighbor lists per window are padded
to 128-slot tiles. Each core processes its windows sorted by tile count
(descending), so the SPMD-uniform per-position tile count T_j = max over
cores of similarly-ranked windows (minimal padding); the host unpermutes
the output rows.

Key trick: the per-query attention bias c[q] = s_emb@W2 + r_emb@W3 + b is
folded into the z-matmul operand on the host: emxT ships em + c[q] @ W1^-1
per neighbor, so (em + delta) @ W1 = em @ W1 + c[q] and the one-hot bias
matmul disappears. The agg-path copy (emx) ships the raw em values plus a
ones column, so one N=257 matmul accumulates agg and den together.

Host prep (pure data layout + tiny dense algebra, inside kernel()):
  emx  f16 [128, NT*264]  raw neighbor embeddings, agg layout, ones col
  emxT f16 [128, NT*256]  (em + delta[q]) pre-transposed for the z-matmul
  P    f16 [128, NT*128]  per-slot segment one-hot
  srx  f32 [SPC, 512]     per-segment [s_emb | r_emb] rows, pre-masked;
                          written to out[:, 256:768] by DRAM->DRAM DMA

Device per window position j:
  em16/emT16/P <- streamed (HWDGE, fp16)
  z     = emT16 @ W1               (PE fp16, 2 matmuls, c included)
  H     = tanh(z)                  (ACT, 4-tile groups, fp16 out)
  score = sum_h H*v                (DVE: tensor_tensor mult + add-tree)
  e     = exp(score)               (ACT, per group)
  wm    = P * e                    (DVE tensor_scalar_mul, fp16)
  agg|den += wm.T @ [em|1]         (PE fp16, single N=257 matmul -> PSUM)
  out[:, 0:256]   = agg / (den + empty)  (DVE recip + ACT scale-copy)
  out[:, 256:768] = srx                  (DRAM->DRAM, no compute)
"""

import os
import sys

import numpy as np

H = 256
EMW = 264  # em row width in the agg layout: 256 em + 1 ones + 7 pad
SEQ_LEN = 10
NCORES = 8
WIN = 128  # segments per window (PSUM partition dim)


def _build_core_shard(c, nbr_ids, seg_ids, QPC, NW):
    """Slice this core's neighbors; per-window counts."""
    seg_lo = c * QPC * SEQ_LEN
    seg_hi = (c + 1) * QPC * SEQ_LEN
    lo = np.searchsorted(seg_ids, seg_lo, "left")
    hi = np.searchsorted(seg_ids, seg_hi, "left")
    segs = (seg_ids[lo:hi] - seg_lo).astype(np.int64)  # 0 .. SPC-1
    nbrs = nbr_ids[lo:hi].astype(np.int64)
    wb = [np.searchsorted(segs, w * WIN, "left") for w in range(NW + 1)]
    cnts = [wb[w + 1] - wb[w] for w in range(NW)]
    return segs, nbrs, wb, cnts


def kernel(s, r, nbr_ids, seg_ids, ent_embeds, rel_embeds, W_attn, b_attn, v_s):
    sys.path.insert(0, "/opt/trn_rl_repo")
    import concourse.bass as bass  # noqa: F401
    import concourse.tile as tile
    from concourse import bacc, mybir
    from concourse.bass_utils import run_bass_kernel_spmd
    from contextlib import ExitStack

    f32 = mybir.dt.float32
    f16 = mybir.dt.float16
    AF = mybir.ActivationFunctionType
    OP = mybir.AluOpType

    s = np.asarray(s)
    r = np.asarray(r)
    nbr_ids = np.asarray(nbr_ids)
    seg_ids = np.asarray(seg_ids)
    ent_embeds = np.ascontiguousarray(np.asarray(ent_embeds, dtype=np.float32))
    rel_embeds = np.ascontiguousarray(np.asarray(rel_embeds, dtype=np.float32))
    W_attn = np.asarray(W_attn, dtype=np.float32)
    b_attn = np.asarray(b_attn, dtype=np.float32)
    v_s = np.asarray(v_s, dtype=np.float32).reshape(-1)

    B = s.shape[0]
    NUM_SEG = B * SEQ_LEN
    QPC = B // NCORES
    SPC = QPC * SEQ_LEN
    NW = SPC // WIN

    ent16 = ent_embeds.astype(np.float16)
    W1 = W_attn[0:256]

    # per-query bias folded through W1^-1 (see module docstring)
    c_all = ent_embeds[s] @ W_attn[256:512] + rel_embeds[r] @ W_attn[512:768] \
        + b_attn                                     # [B, 256]
    delta = (c_all @ np.linalg.inv(W1)).astype(np.float32)

    # ---------------- host-side layout ----------------
    shards = [_build_core_shard(c, nbr_ids, seg_ids, QPC, NW) for c in range(NCORES)]
    tc_cw = np.array(
        [[max(1, -(-shards[c][3][w] // 128)) for w in range(NW)]
         for c in range(NCORES)])
    perm = [list(np.argsort(-tc_cw[c], kind="stable")) for c in range(NCORES)]
    T_j = [int(max(tc_cw[c][perm[c][j]] for c in range(NCORES)))
           for j in range(NW)]
    tb = np.concatenate([[0], np.cumsum(T_j)]).astype(np.int64)
    NT = int(tb[-1])
    TMAX = max(T_j)

    counts_all = np.bincount(seg_ids.astype(np.int64), minlength=NUM_SEG)

    in_maps = []
    for c in range(NCORES):
        segs, nbrs, wb, cnts = shards[c]

        em_idx = np.full((NT, 128), -1, dtype=np.int64)
        segl = np.full((NT, 128), -1, dtype=np.int64)
        qglob = np.full((NT, 128), -1, dtype=np.int64)
        invw = np.zeros((128, NW), dtype=np.float32)

        for j in range(NW):
            w = perm[c][j]
            cnt = cnts[w]
            flat_lo = int(tb[j]) * 128
            sl = slice(wb[w], wb[w + 1])
            idx_flat = np.arange(flat_lo, flat_lo + cnt)
            em_idx.reshape(-1)[idx_flat] = nbrs[sl]
            segl.reshape(-1)[idx_flat] = segs[sl] - w * WIN
            qglob.reshape(-1)[idx_flat] = segs[sl] // SEQ_LEN + c * QPC
            invw[:, j] = (
                counts_all[c * SPC + w * WIN : c * SPC + (w + 1) * WIN] == 0
            ).astype(np.float32)

        E = ent16[np.maximum(em_idx, 0)]           # [NT, 128, 256] raw f16
        E[em_idx < 0] = 0
        emx = np.zeros((NT, 128, EMW), dtype=np.float16)
        emx[:, :, 0:H] = E
        emx[:, :, H] = 1.0
        emx = np.ascontiguousarray(emx.transpose(1, 0, 2).reshape(128, NT * EMW))

        Eaug = E.astype(np.float32) + np.where(
            (qglob >= 0)[:, :, None], delta[np.maximum(qglob, 0)], 0.0)
        Eaug = Eaug.astype(np.float16)
        emxT = np.ascontiguousarray(
            Eaug.reshape(NT, 128, 2, 128).transpose(3, 0, 2, 1)
            .reshape(128, NT * H))

        # P: per-slot segment one-hot [128, NT*128]
        P = np.zeros((NT, 128, 128), dtype=np.float16)
        tt, pp = np.nonzero(segl >= 0)
        P[tt, pp, segl[tt, pp]] = 1.0
        P = np.ascontiguousarray(P.transpose(1, 0, 2).reshape(128, NT * 128))

        # per-segment [s_emb | r_emb] rows, masked (original row order)
        segq = np.arange(SPC) // SEQ_LEN + c * QPC
        mask = (counts_all[c * SPC : (c + 1) * SPC] > 0).astype(np.float32)[:, None]
        srx = np.empty((SPC, 2 * H), dtype=np.float32)
        srx[:, 0:H] = ent_embeds[s[segq]] * mask
        srx[:, H : 2 * H] = rel_embeds[r[segq]] * mask

        im = {
            "emx": emx,
            "emxT": emxT,
            "pmat": P,
            "srx": srx,
            "wq1": W1.reshape(2, 128, H).transpose(1, 0, 2)
                   .astype(np.float16).copy(),
            "vbc": np.tile(v_s.astype(np.float16), (128, 4, 1))
                   .reshape(128, 4 * H),
            "invw": invw,
        }
        in_maps.append(im)

    # ---------------- build the SPMD program ----------------
    print("[kernel] host prep done", flush=True)
    nc = bacc.Bacc("TRN2", target_bir_lowering=False, debug=False,
                   num_devices=NCORES)

    def din(name, shape, dt):
        return nc.dram_tensor(name, shape, dt, kind="ExternalInput").ap()

    emx_ap = din("emx", [128, NT * EMW], f16)
    emxT_ap = din("emxT", [128, NT * H], f16)
    pmat_ap = din("pmat", [128, NT * 128], f16)
    srx_ap = din("srx", [SPC, 2 * H], f32)
    wq1_ap = din("wq1", [128, 2, H], f16)
    vbc_ap = din("vbc", [128, 4 * H], f16)
    invw_ap = din("invw", [128, NW], f32)
    out_ap = nc.dram_tensor("out", [SPC, 3 * H], f32, kind="ExternalOutput").ap()

    import time as _time
    _t0 = _time.time()
    with tile.TileContext(nc) as tc, ExitStack() as ctx:
        cons = ctx.enter_context(tc.tile_pool(name="cons", bufs=1))
        emp = ctx.enter_context(tc.tile_pool(name="emp", bufs=3))
        emq = ctx.enter_context(tc.tile_pool(name="emq", bufs=3))
        pp_ = ctx.enter_context(tc.tile_pool(name="pp", bufs=2))
        wk = ctx.enter_context(tc.tile_pool(name="wk", bufs=2))
        wmp = ctx.enter_context(tc.tile_pool(name="wmp", bufs=4))
        op = ctx.enter_context(tc.tile_pool(name="op", bufs=2))
        psz = ctx.enter_context(tc.tile_pool(name="psz", bufs=2, space="PSUM"))
        psa = ctx.enter_context(tc.tile_pool(name="psa", bufs=2, space="PSUM"))

        def cload(tag, shape, dt, ap):
            t = cons.tile(shape, dt, tag=tag)
            nc.sync.dma_start(t[:], ap[:])
            return t

        wq1 = cload("wq1", [128, 2, H], f16, wq1_ap)
        vbc4_flat = cload("vbc", [128, 4 * H], f16, vbc_ap)
        vbc4 = vbc4_flat.rearrange("p (t h) -> p t h", t=4)
        invw = cload("invw", [128, NW], f32, invw_ap)

        # ---- main loop over window positions ----
        NW_RUN = int(os.environ.get("KERNEL_NWIN", str(NW)))
        for j in range(NW_RUN):
            TW = T_j[j]
            base = int(tb[j])

            em16 = emp.tile([128, TMAX, EMW], f16, tag="em")
            emT16 = emq.tile([128, 2 * TMAX, 128], f16, tag="emT")
            halves = [(0, TW // 2), (TW // 2, TW)]
            for (lo, hi) in halves:
                if hi <= lo:
                    continue
                nc.sync.dma_start(
                    em16[:, lo:hi, :],
                    emx_ap[:, (base + lo) * EMW : (base + hi) * EMW])
                nc.scalar.dma_start(
                    emT16[:, 2 * lo : 2 * hi, :],
                    emxT_ap[:, (base + lo) * H : (base + hi) * H])
            P_w = pp_.tile([128, TMAX * 128], f16, tag="P")
            nc.sync.dma_start(P_w[:, 0 : TW * 128],
                              pmat_ap[:, base * 128 : (base + TW) * 128])

            scores = wk.tile([128, TMAX], f32, tag="sc")
            ebuf = wk.tile([128, TMAX], f32, tag="eb")

            ngrp = -(-TW // 4)
            for g in range(ngrp):
                t0 = g * 4
                nt = min(4, TW - t0)
                zp = psz.tile([128, 4, H], f32, tag="z")
                for tg in range(nt):
                    t = t0 + tg
                    zps = zp[:, tg, :]
                    nc.tensor.matmul(zps, emT16[:, 2 * t, :], wq1[:, 0, :],
                                     start=True, stop=False)
                    nc.tensor.matmul(zps, emT16[:, 2 * t + 1, :], wq1[:, 1, :],
                                     start=False, stop=True)
                Hsb = wk.tile([128, 4, H], f16, tag="H")
                nc.scalar.activation(Hsb[:, 0:nt, :], zp[:, 0:nt, :], AF.Tanh)
                hv = wk.tile([128, 4, H], f16, tag="hv")
                hv2 = wk.tile([128, 4, 128], f16, tag="hv2")
                nc.vector.tensor_tensor(hv[:, 0:nt, :], Hsb[:, 0:nt, :],
                                        vbc4[:, 0:nt, :], OP.mult)
                nc.vector.tensor_tensor(hv2[:, 0:nt, :], hv[:, 0:nt, 0:128],
                                        hv[:, 0:nt, 128:256], OP.add)
                nc.vector.tensor_tensor(hv[:, 0:nt, 0:64], hv2[:, 0:nt, 0:64],
                                        hv2[:, 0:nt, 64:128], OP.add)
                nc.vector.tensor_tensor(hv2[:, 0:nt, 0:32], hv[:, 0:nt, 0:32],
                                        hv[:, 0:nt, 32:64], OP.add)
                nc.vector.reduce_sum(scores[:, t0 : t0 + nt],
                                     hv2[:, 0:nt, 0:32],
                                     axis=mybir.AxisListType.X)
                nc.scalar.activation(ebuf[:, t0 : t0 + nt],
                                     scores[:, t0 : t0 + nt], AF.Exp)

            agg = psa.tile([128, EMW], f32, tag="agg")
            for t in range(TW):
                wm = wmp.tile([128, 128], f16, tag="wm")
                nc.vector.tensor_scalar_mul(
                    wm[:], P_w[:, t * 128 : (t + 1) * 128],
                    ebuf[:, t : t + 1])
                nc.tensor.matmul(agg[:, 0:257], wm[:], em16[:, t, 0:257],
                                 start=(t == 0), stop=(t == TW - 1))

            dtmp = wk.tile([128, 1], f32, tag="dtmp")
            nc.vector.tensor_add(dtmp[:], agg[:, 256:257], invw[:, j : j + 1])
            dinv = wk.tile([128, 1], f32, tag="dinv")
            nc.vector.reciprocal(dinv[:], dtmp[:])

            out_sb = op.tile([128, 256], f32, tag="out")
            nc.scalar.activation(out_sb[:], agg[:, 0:256], AF.Copy,
                                 scale=dinv[:])
            nc.sync.dma_start(out_ap[j * 128 : (j + 1) * 128, 0:256],
                              out_sb[:])

            if j % 5 == 2:
                rlo = (j - 2) * 128
                rhi = min(rlo + 5 * 128, SPC)
                nc.scalar.dma_start(out_ap[rlo:rhi, 256:768],
                                    srx_ap[rlo:rhi, :])

    print(f"[kernel] program built+scheduled in {_time.time()-_t0:.1f}s",
          flush=True)
    nc.compile()
    print("[kernel] bacc.compile done; launching", flush=True)

    def assemble(core_outs):
        full = np.empty((NCORES * SPC, 3 * H), dtype=np.float32)
        for c in range(NCORES):
            o = core_outs[c]
            blk = full[c * SPC : (c + 1) * SPC]
            blk[:, 256:768] = o[:, 256:768]
            for j in range(NW):
                w = perm[c][j]
                blk[w * WIN : (w + 1) * WIN, 0:256] = \
                    o[j * WIN : (j + 1) * WIN, 0:256]
        return full.reshape(B, SEQ_LEN, 3 * H)

    if os.environ.get("KERNEL_SIM"):
        from concourse.bass_interp import CoreSim
        sim = CoreSim(nc, trace=False)
        for k, v in in_maps[0].items():
            sim.tensor(k)[:] = v
        sim.simulate(check_with_hw=False)
        print("[kernel] CoreSim passed", flush=True)
        return assemble([np.array(sim.tensor("out"))] * NCORES)

    trace = bool(int(os.environ.get("KERNEL_TRACE", "0")))
    if trace:
        _install_prof_hook()
    res = run_bass_kernel_spmd(nc, in_maps, list(range(NCORES)), trace=trace)
    if trace and res.exec_time_ns is not None:
        print(f"HW exec time: {res.exec_time_ns} ns")

    return assemble([res.results[c]["out"] for c in range(NCORES)])


def _install_prof_hook():
    """Shim antenv.axon_hooks so trace=True can NTFF-profile under axon."""
    import contextlib
    import ctypes
    import types

    import antenv

    if "antenv.axon_hooks" in sys.modules:
        return
    so = "/opt/axon/libaxon_pjrt.so"
    lib = ctypes.CDLL(so)
    if not hasattr(lib, "axon_start_nrt_profile"):
        return
    lib.axon_start_nrt_profile.argtypes = [ctypes.POINTER(ctypes.c_int64),
                                           ctypes.c_size_t]
    lib.axon_start_nrt_profile.restype = ctypes.c_int64
    lib.axon_stop_nrt_profile.argtypes = [ctypes.c_char_p]
    lib.axon_stop_nrt_profile.restype = ctypes.c_int64

    @contextlib.contextmanager
    def _hook(output_dir, device_ids):
        import jax

        jax.devices()
        if device_ids:
            ids = (ctypes.c_int64 * len(device_ids))(*device_ids)
            rc = lib.axon_start_nrt_profile(ids, len(device_ids))
        else:
            rc = lib.axon_start_nrt_profile(None, 0)
        if rc != 0:
            raise RuntimeError(f"axon_start_nrt_profile rc={rc}")
        try:
            yield
        finally:
            n = lib.axon_stop_nrt_profile(str(output_dir).encode())
            print(f"profile: {n} file(s) written to {output_dir}",
                  file=sys.stderr)

    mod = types.ModuleType("antenv.axon_hooks")
    mod.get_axon_ntff_profile_hook = lambda: _hook
    mod.set_axon_ntff_profile_hook = lambda h: None
    sys.modules["antenv.axon_hooks"] = mod
    antenv.axon_hooks = mod
